# revision 9
# baseline (speedup 1.0000x reference)
"""MultiHeadAttnBlock TRN2 kernel, v4.

Sharding: core i = (batch b = i//4, query quarter sq = i%4).

Structure (engines in steady state):
 - PE: scores (K=64, two heads at row halves) + attn@v, 8 N=512 matmuls per
   key-tile; projections k/v interleaved so LDWEIGHTS hides under streams.
 - ACT: exp of score columns [0:512] of each head tile (table exp with the
   1/SCHRAUD_A descale in the free activation scale slot).
 - DVE: score columns [512:1024] via a Schraudolph bit-exp: scores arrive
   pre-scaled by SCHRAUD_A (folded into wq on host), so the op is a single
   round(x + B) into int16, bitcast to bf16.
 - sc PSUM is four one-bank tiles (per head x per 512-column chunk) and the
   e tile is split per engine, so each scores->exp->scores sub-pipeline
   cycles in ~960ns against a ~1750ns iteration - no WAR stalls.
 - group-norm affine is folded into the projection weights; k-bias is
   softmax-invariant (dropped), v-bias routed through wo into a final
   column, q-bias kept via a tiny wq@By matmul.
 - x stats on DVE bn_stats; y stats via ACT squares + PE pooling matmuls.
 - denominator: ones-column of v'; reciprocal via 32-lane DMA spread; the
   partition broadcast is a K=1 ones matmul into drained PSUM (no DRAM
   round trip).
"""

import numpy as np
import ml_dtypes

import concourse.bass as bass
import concourse.mybir as mybir
import bass_rust as _br
from concourse.tile import TileContext
from concourse.bass_utils import run_bass_kernel_spmd

F32 = mybir.dt.float32
BF16 = mybir.dt.bfloat16
I16 = mybir.dt.int16
AF = mybir.ActivationFunctionType
OP = mybir.AluOpType

C = 256
S = 4096
SQ = 1024
H = 4
D = 64
G = 32
EPS = 1e-6
NT = 32
VW = D + 2

SCHRAUD_A = 184.6650292
SCHRAUD_B = 16251.44
XA = 512   # exp column split: ACT [0:XA], DVE [XA:1024]


def build_nc():
    nc = bass.Bass("TRN2", target_bir_lowering=False, debug=False, num_devices=8)

    def din(name, shape, dt=F32):
        return nc.dram_tensor(name, shape, dt, kind="ExternalInput").ap()

    x_d = din("x", [C, S], BF16)
    y_d = din("y", [C, S], BF16)
    xq_d = din("xq", [C, SQ])
    wall_d = din("wall", [C, 4 * C], BF16)   # (wk|wv|wq|wo).T halves
    vp_d = din("vp", [C, 6 + G])             # (vecs|pool)
    pooly_d = din("poolym", [C, G], BF16)
    exp_d = din("expandm", [G, C])
    out_d = nc.dram_tensor("out", [C, SQ], F32, kind="ExternalOutput").ap()

    with TileContext(nc) as tc:
        with (
            tc.tile_pool(name="pers", bufs=1) as pers,
            tc.tile_pool(name="sb1", bufs=1) as sb1,
            tc.tile_pool(name="sb2", bufs=2) as sb2,
            tc.tile_pool(name="expp", bufs=2) as expp,
            tc.tile_pool(name="ps", bufs=1, space="PSUM") as ps,
        ):
            # ---- persistent tiles -------------------------------------
            xf = [pers.tile([128, S], BF16, tag=f"xf{m}", name=f"xf{m}")
                  for m in range(2)]
            yf = [pers.tile([128, S], BF16, tag=f"yf{m}", name=f"yf{m}")
                  for m in range(2)]
            xq = [pers.tile([128, SQ], F32, tag=f"xq{m}", name=f"xq{m}")
                  for m in range(2)]
            k_sb = [[pers.tile([128, 1024], BF16, tag=f"ksb{m}_{j}",
                               name=f"ksb{m}_{j}") for j in range(4)]
                    for m in range(2)]
            q_sb = [pers.tile([128, SQ], BF16, tag=f"qsb{m}", name=f"qsb{m}")
                    for m in range(2)]
            v_sb = [pers.tile([128, 8 * H * VW], BF16, tag=f"vsb{j}",
                              name=f"vsb{j}") for j in range(4)]
            out_ds = [pers.tile([128, SQ], BF16, tag=f"ods{m}", name=f"ods{m}")
                      for m in range(2)]
            wall = [pers.tile([128, 4 * C], BF16, tag=f"wall{m}",
                              name=f"wall{m}") for m in range(2)]
            wk_b = [wall[m][:, 0:C] for m in range(2)]
            wv_b = [wall[m][:, C:2 * C] for m in range(2)]
            wq_b = [wall[m][:, 2 * C:3 * C] for m in range(2)]
            wo_b = [wall[m][:, 3 * C:4 * C] for m in range(2)]
            wqf = [pers.tile([128, C], BF16, tag=f"wqf{m}", name=f"wqf{m}")
                   for m in range(2)]
            wkf = [pers.tile([128, C], BF16, tag=f"wkf{m}", name=f"wkf{m}")
                   for m in range(2)]
            wvf = [pers.tile([128, C], BF16, tag=f"wvf{m}", name=f"wvf{m}")
                   for m in range(2)]
            vp = [pers.tile([128, 6 + G], F32, tag=f"vp{m}", name=f"vp{m}")
                  for m in range(2)]
            _vc = {"bq8": 0, "bo2": 1, "g1": 2, "b1": 3, "g2": 4, "b2": 5}
            gb = {nm: [vp[m][:, i:i + 1] for m in range(2)]
                  for nm, i in _vc.items()}
            den32 = [pers.tile([32, 32], F32, tag=f"den32_{hh}",
                               name=f"den32_{hh}") for hh in range(2)]
            rc32 = [pers.tile([32, 32], F32, tag=f"rc32_{hh}",
                              name=f"rc32_{hh}") for hh in range(2)]
            rc32b = [pers.tile([32, 32], BF16, tag=f"rc32b_{hh}",
                               name=f"rc32b_{hh}") for hh in range(2)]
            rrow = [pers.tile([1, SQ], BF16, tag=f"rrow{hh}", name=f"rrow{hh}")
                    for hh in range(2)]
            ones1 = pers.tile([1, D], BF16, tag="ones1", name="ones1")
            colq = [pers.tile([128, 1], F32, tag=f"colq{m}", name=f"colq{m}")
                    for m in range(2)]
            colc = [pers.tile([128, 1], F32, tag=f"colc{m}", name=f"colc{m}")
                    for m in range(2)]
            bxb = [pers.tile([128, 1], BF16, tag=f"bxb{m}", name=f"bxb{m}")
                   for m in range(2)]
            byb = [pers.tile([128, 1], BF16, tag=f"byb{m}", name=f"byb{m}")
                   for m in range(2)]
            bvx_sb = [pers.tile([128, 1], BF16, tag=f"bvx{m}", name=f"bvx{m}")
                      for m in range(2)]
            pool_sb = [vp[m][:, 6:6 + G] for m in range(2)]
            pooly_sb = [pers.tile([128, G], BF16, tag=f"ply{m}",
                                  name=f"ply{m}") for m in range(2)]
            expand_sb = pers.tile([G, C], F32, tag="ex", name="ex")

            nc.gpsimd.memset(ones1[:], 1.0)
            for j in range(4):
                vview = v_sb[j][:].rearrange("p (t h e) -> p t h e", t=8, h=H)
                nc.gpsimd.memset(vview[:, :, :, D:D + 2], 1.0)

            # ---- stage 1: DMA + stats ---------------------------------
            s6x = [sb1.tile([128, 48], F32, tag=f"s6x{m}", name=f"s6x{m}")
                   for m in range(2)]
            # x first (8 chunk DMAs, issue cost ~0.6us each on Sync):
            # DVE bn_stats chases the DMA
            for m in range(2):
                cs = slice(m * 128, (m + 1) * 128)
                for ch in range(4):
                    fs = slice(ch * 1024, (ch + 1) * 1024)
                    nc.sync.dma_start(out=xf[m][:, fs], in_=x_d[cs, fs])
                    for h2 in range(2):
                        c8 = 2 * ch + h2
                        nc.vector.bn_stats(
                            s6x[m][:, c8 * 6:(c8 + 1) * 6],
                            xf[m][:, c8 * 512:(c8 + 1) * 512])
            # small constants (tiny transfers; needed from ~12us)
            for m in range(2):
                nc.sync.dma_start(out=vp[m][:],
                                  in_=vp_d[m * 128:(m + 1) * 128, :])
                nc.sync.dma_start(out=pooly_sb[m][:],
                                  in_=pooly_d[m * 128:(m + 1) * 128, :])
            nc.sync.dma_start(out=expand_sb[:], in_=exp_d[:])
            # packed weights (needed at ~14us for the folds)
            for m in range(2):
                cs = slice(m * 128, (m + 1) * 128)
                for half in range(2):
                    nc.sync.dma_start(
                        out=wall[m][:, half * 512:(half + 1) * 512],
                        in_=wall_d[cs, half * 512:(half + 1) * 512])
            # y second: ACT squares + PE pooling
            gy1 = ps.tile([G, 512], F32, tag="bA", padded_shape=[128, 512],
                          name="gy1")
            gy2 = ps.tile([G, 512], F32, tag="bB", padded_shape=[128, 512],
                          name="gy2")
            n1 = n2 = 0
            for ch in range(4):
                fs = slice(ch * 1024, (ch + 1) * 1024)
                for m in range(2):
                    cs = slice(m * 128, (m + 1) * 128)
                    nc.sync.dma_start(out=yf[m][:, fs], in_=y_d[cs, fs])
                for m in range(2):
                    for h2 in range(2):
                        f5 = slice(ch * 1024 + h2 * 512,
                                   ch * 1024 + (h2 + 1) * 512)
                        nc.tensor.matmul(gy1[:], lhsT=pooly_sb[m][:],
                                         rhs=yf[m][:, f5],
                                         start=(n1 == 0), stop=(n1 == 15))
                        n1 += 1
                        ysq = sb1.tile([128, 512], BF16,
                                       tag=f"ysq{n2 % 4}",
                                       name=f"ysqt{ch}_{m}_{h2}")
                        nc.scalar.square(ysq[:], yf[m][:, f5])
                        nc.tensor.matmul(gy2[:], lhsT=pooly_sb[m][:],
                                         rhs=ysq[:],
                                         start=(n2 == 0), stop=(n2 == 15))
                        n2 += 1
            for m in range(2):
                nc.sync.dma_start(out=xq[m][:],
                                  in_=xq_d[m * 128:(m + 1) * 128, :])

            def affine_tail(gs, gamma, beta, tagp):
                """gs [G,2] = (mu_g, E[x^2]_g) -> per-channel A,B [128,1]."""
                ve = nc.vector
                musq = sb1.tile([G, 1], F32, tag=f"gmusq{tagp}",
                                name=f"gmusq{tagp}")
                ve.tensor_mul(musq[:], gs[:, 0:1], gs[:, 0:1])
                veps = sb1.tile([G, 1], F32, tag=f"veps{tagp}",
                                name=f"veps{tagp}")
                ve.tensor_sub(veps[:], gs[:, 1:2], musq[:])
                ve.tensor_scalar_add(veps[:], veps[:], EPS)
                sq = sb1.tile([G, 1], F32, tag=f"gsq{tagp}", name=f"gsq{tagp}")
                nc.scalar.sqrt(sq[:], veps[:])
                r0 = sb1.tile([G, 1], F32, tag=f"gr0{tagp}", name=f"gr0{tagp}")
                nc.vector.reciprocal(r0[:], sq[:])
                y2 = sb1.tile([G, 1], F32, tag=f"gy2{tagp}", name=f"gy2{tagp}")
                ve.tensor_mul(y2[:], r0[:], r0[:])
                ve.tensor_mul(y2[:], veps[:], y2[:])
                ve.tensor_scalar(y2[:], y2[:], -0.5, 1.5, OP.mult, OP.add)
                gs2 = sb1.tile([G, 2], F32, tag=f"gs2{tagp}",
                               name=f"gs2{tagp}")
                ve.tensor_mul(gs2[:, 0:1], r0[:], y2[:])
                ve.tensor_copy(gs2[:, 1:2], gs[:, 0:1])
                A, B = [], []
                for m in range(2):
                    pc = ps.tile([128, 2], F32, tag="bD",
                                 padded_shape=[128, 512], name=f"pc{tagp}{m}")
                    nc.tensor.matmul(
                        pc[:], lhsT=expand_sb[:, m * 128:(m + 1) * 128],
                        rhs=gs2[:], start=True, stop=True)
                    a = sb1.tile([128, 1], F32, tag=f"A{tagp}{m}",
                                 name=f"A{tagp}{m}")
                    nc.vector.tensor_mul(a[:], pc[:, 0:1], gamma[m])
                    bmid = sb1.tile([128, 1], F32, tag=f"Bm{tagp}{m}",
                                    name=f"Bm{tagp}{m}")
                    nc.vector.tensor_mul(bmid[:], pc[:, 1:2], a[:])
                    b_ = sb1.tile([128, 1], F32, tag=f"B{tagp}{m}",
                                  name=f"B{tagp}{m}")
                    ve.tensor_sub(b_[:], beta[m], bmid[:])
                    A.append(a)
                    B.append(b_)
                return A, B

            # ---- x affine + folded wk/wv ------------------------------
            stats_c = []
            for m in range(2):
                mv = sb1.tile([128, 2], F32, tag=f"mvx{m}", name=f"mvx{m}")
                nc.vector.bn_aggr(mv[:], s6x[m][:])
                st = sb1.tile([128, 2], F32, tag=f"stx{m}", name=f"stx{m}")
                nc.vector.tensor_copy(st[:, 0:1], mv[:, 0:1])
                msq = sb1.tile([128, 1], F32, tag=f"msqx{m}", name=f"msqx{m}")
                nc.vector.tensor_mul(msq[:], mv[:, 0:1], mv[:, 0:1])
                nc.vector.tensor_add(st[:, 1:2], mv[:, 1:2], msq[:])
                stats_c.append(st)
            gpx = ps.tile([G, 2], F32, tag="bC", padded_shape=[128, 512],
                          name="gpx")
            for m in range(2):
                nc.tensor.matmul(gpx[:], lhsT=pool_sb[m][:],
                                 rhs=stats_c[m][:],
                                 start=(m == 0), stop=(m == 1))
            gsx = sb1.tile([G, 2], F32, tag="gsx", name="gsx")
            nc.vector.tensor_copy(gsx[:], gpx[:])
            Ax, Bx = affine_tail(gsx, gb["g1"], gb["b1"], "x")
            for m in range(2):
                nc.vector.tensor_scalar_mul(wkf[m][:], wk_b[m][:],
                                            Ax[m][:, 0:1])
                nc.vector.tensor_scalar_mul(wvf[m][:], wv_b[m][:],
                                            Ax[m][:, 0:1])
                nc.vector.tensor_copy(bxb[m][:], Bx[m][:])

            # ---- k + v projections, interleaved (LDW hiding) ----------
            kjobs = [("k", m, n) for m in range(2) for n in range(0, S, 512)]
            vjobs = [("v", t, 0) for t in range(NT)]
            order = []
            while kjobs or vjobs:
                if kjobs:
                    order.append(kjobs.pop(0))
                for _ in range(2):
                    if vjobs:
                        order.append(vjobs.pop(0))
            nk = nv = 0
            for job in order:
                if job[0] == "k":
                    _, m, n = job
                    pk = ps.tile([128, 512], F32,
                                 tag="bA" if nk % 2 == 0 else "bB",
                                 padded_shape=[128, 512], name=f"pk{m}_{n}")
                    for kk in range(2):
                        nc.tensor.matmul(
                            pk[:], lhsT=wkf[kk][:, m * 128:(m + 1) * 128],
                            rhs=xf[kk][:, n:n + 512],
                            start=(kk == 0), stop=(kk == 1))
                    kdst = k_sb[m][n // 1024][:, n % 1024:n % 1024 + 512]
                    if nk % 2 == 0:
                        nc.scalar.copy(kdst, pk[:])
                    else:
                        nc.vector.tensor_copy(kdst, pk[:])
                    nk += 1
                else:
                    _, t, _ = job
                    pv = ps.tile([128, C], F32,
                                 tag="bC" if nv % 2 == 0 else "bD",
                                 padded_shape=[128, 512], name=f"pv{t}")
                    tsl = slice(t * 128, (t + 1) * 128)
                    for kk in range(2):
                        nc.tensor.matmul(pv[:], lhsT=xf[kk][:, tsl],
                                         rhs=wvf[kk][:],
                                         start=(kk == 0), stop=(kk == 1))
                    pvv = pv[:].rearrange("p (h e) -> p h e", h=H)
                    dst = v_sb[t // 8][:, (t % 8) * H * VW:
                                       (t % 8 + 1) * H * VW]
                    dvv = dst.rearrange("p (h e) -> p h e", h=H)[:, :, 0:D]
                    if nv % 2 == 0:
                        nc.scalar.copy(dvv, pvv)
                    else:
                        nc.vector.tensor_copy(dvv, pvv)
                    nv += 1

            # ---- y affine + folded wq + q projection ------------------
            gsy = sb1.tile([G, 2], F32, tag="gsy", name="gsy")
            nc.vector.tensor_reduce(gsy[:, 0:1], gy1[:],
                                    mybir.AxisListType.X, OP.add)
            nc.vector.tensor_reduce(gsy[:, 1:2], gy2[:],
                                    mybir.AxisListType.X, OP.add)
            Ay, By = affine_tail(gsy, gb["g2"], gb["b2"], "y")
            for m in range(2):
                nc.vector.tensor_scalar_mul(wqf[m][:], wq_b[m][:],
                                            Ay[m][:, 0:1])
                nc.vector.tensor_copy(byb[m][:], By[m][:])
            pbq = [ps.tile([128, 1], F32, tag="bC", padded_shape=[128, 512],
                           name=f"pbq{mh}") for mh in range(2)]
            for mh in range(2):
                for kk in range(2):
                    nc.tensor.matmul(pbq[mh][:],
                                     lhsT=wq_b[kk][:, mh * 128:(mh + 1) * 128],
                                     rhs=byb[kk][:],
                                     start=(kk == 0), stop=(kk == 1))
                nc.vector.tensor_add(colq[mh][:], pbq[mh][:], gb["bq8"][mh])
            pqs = []
            for m in range(2):
                pq = ps.tile([128, SQ], F32, tag="dE" if m == 0 else "dF",
                             name=f"pq{m}")
                for n in range(0, SQ, 512):
                    for kk in range(2):
                        nc.tensor.matmul(
                            pq[:, n:n + 512],
                            lhsT=wqf[kk][:, m * 128:(m + 1) * 128],
                            rhs=yf[kk][:, n:n + 512],
                            start=(kk == 0), stop=(kk == 1))
                pqs.append(pq)
            for m in range(2):
                nc.vector.tensor_scalar_add(q_sb[m][:], pqs[m][:],
                                            colq[m][:, 0:1])

            # v bias chain: colc[mo] = wo@(wv@Bx) + bo2
            pbv = [ps.tile([128, 1], F32, tag="bC", padded_shape=[128, 512],
                           name=f"pbv{mh}") for mh in range(2)]
            for mh in range(2):
                for kk in range(2):
                    nc.tensor.matmul(pbv[mh][:],
                                     lhsT=wv_b[kk][:, mh * 128:(mh + 1) * 128],
                                     rhs=bxb[kk][:],
                                     start=(kk == 0), stop=(kk == 1))
                nc.vector.tensor_copy(bvx_sb[mh][:], pbv[mh][:])
            pbc = [ps.tile([128, 1], F32, tag="bD", padded_shape=[128, 512],
                           name=f"pbc{mo}") for mo in range(2)]
            for mo in range(2):
                for mh in range(2):
                    nc.tensor.matmul(pbc[mo][:],
                                     lhsT=wo_b[mh][:, mo * 128:(mo + 1) * 128],
                                     rhs=bvx_sb[mh][:],
                                     start=(mh == 0), stop=(mh == 1))
                nc.vector.tensor_add(colc[mo][:], pbc[mo][:], gb["bo2"][mo])

            # ---- stage 3: attention ------------------------------------
            po = []
            for p in range(2):
                sc = [[ps.tile([128, 512], F32,
                               tag=["bA", "bB", "bC", "bD"][2 * hh + ni],
                               padded_shape=[128, 512],
                               name=f"sc{p}_{hh}_{ni}")
                       for ni in range(2)] for hh in range(2)]
                acc = [ps.tile([VW, SQ], F32, tag=["dE", "dF"][hh],
                               padded_shape=[128, 1024],
                               name=f"acc{p}_{hh}") for hh in range(2)]

                def emit_scores(t):
                    tsl = slice((t % 8) * 128, (t % 8 + 1) * 128)
                    for ni in range(2):
                        for hh in range(2):
                            lo = hh * 64
                            n = ni * 512
                            nc.tensor.matmul(
                                sc[hh][ni][:],
                                lhsT=k_sb[p][t // 8][lo:lo + 64, tsl],
                                rhs=q_sb[p][lo:lo + 64, n:n + 512],
                                start=True, stop=True)

                def emit_exp(t):
                    es = []
                    for hh in range(2):
                        ea = expp.tile([128, XA], BF16, tag=f"ea{hh}",
                                       name=f"ea{p}_{hh}_{t}")
                        eb = expp.tile([128, SQ - XA], BF16, tag=f"eb{hh}",
                                       name=f"eb{p}_{hh}_{t}")
                        nc.scalar.activation(ea[:], sc[hh][0][:],
                                             AF.Exp, scale=1.0 / SCHRAUD_A)
                        nc.vector.tensor_scalar_add(
                            eb[:].bitcast(I16), sc[hh][1][:], SCHRAUD_B)
                        es.append((ea, eb))
                    return es

                emit_scores(0)
                for t in range(NT):
                    es = emit_exp(t)
                    if t + 1 < NT:
                        emit_scores(t + 1)
                    for hh in range(2):
                        h = 2 * p + hh
                        voff = (t % 8) * H * VW + h * VW
                        lhsv = v_sb[t // 8][:, voff:voff + VW]
                        for ni in range(2):
                            nc.tensor.matmul(
                                acc[hh][:, ni * 512:(ni + 1) * 512],
                                lhsT=lhsv, rhs=es[hh][ni][:],
                                start=(t == 0), stop=(t == NT - 1))
                # ---- drain ----
                asbs = []
                for hh in range(2):
                    asb = sb2.tile([VW, SQ], F32, tag="asb", name="asb")
                    if hh == 0:
                        nc.vector.tensor_copy(asb[:], acc[hh][:])
                    else:
                        nc.scalar.copy(asb[:], acc[hh][:])
                    asbs.append(asb)
                if p == 1:
                    # wo accumulation on out_ds[0] while the drain DMAs fly
                    # (reuses the just-freed acc banks)
                    for mo in range(2):
                        po_t = ps.tile([128, SQ], F32,
                                       tag="dE" if mo == 0 else "dF",
                                       name=f"po{mo}")
                        po.append(po_t)
                        for n in range(0, SQ, 512):
                            nc.tensor.matmul(
                                po_t[:, n:n + 512],
                                lhsT=wo_b[0][:, mo * 128:(mo + 1) * 128],
                                rhs=out_ds[0][:, n:n + 512],
                                start=True, stop=False)
                for hh in range(2):
                    # gather the 32-lane reciprocal into one bf16 row, then
                    # partition-broadcast via a K=1 ones matmul into drained
                    # PSUM.  Pair 0 uses the acc banks (dE/dF) so pair-1
                    # scores can claim bA..bD immediately.
                    nc.sync.dma_start(out=den32[hh][:],
                                      in_=asbs[hh][D:D + 1, :])
                    nc.vector.reciprocal(rc32[hh][:], den32[hh][:])
                    nc.vector.tensor_copy(rc32b[hh][:], rc32[hh][:])
                    nc.sync.dma_start(out=rrow[hh][:], in_=rc32b[hh][:])
                    if p == 0:
                        rbt = ps.tile([D, SQ], F32, tag=["dE", "dF"][hh],
                                      name=f"rb{p}{hh}")
                        rbs = [rbt[:, 0:512], rbt[:, 512:1024]]
                    else:
                        rbs = [ps.tile([D, 512], F32,
                                       tag=["bA", "bB", "bC", "bD"][2*hh+ni],
                                       padded_shape=[128, 512],
                                       name=f"rb{p}{hh}{ni}")[:]
                               for ni in range(2)]
                    for ni in range(2):
                        ns = slice(ni * 512, (ni + 1) * 512)
                        nc.tensor.matmul(rbs[ni], lhsT=ones1[:],
                                         rhs=rrow[hh][:, ns],
                                         start=True, stop=True)
                        if hh == 0:
                            nc.vector.tensor_mul(out_ds[p][0:64, ns],
                                                 asbs[hh][0:D, ns], rbs[ni])
                        else:
                            hsh = sb2.tile([64, 512], BF16, tag="hsh",
                                           name="hsh")
                            nc.vector.tensor_mul(hsh[:], asbs[hh][0:D, ns],
                                                 rbs[ni])
                            nc.sync.dma_start(out=out_ds[p][64:128, ns],
                                              in_=hsh[:])

            # ---- stage 4: output projection + residual -----------------
            for mo in range(2):
                for n in range(0, SQ, 512):
                    nc.tensor.matmul(
                        po[mo][:, n:n + 512],
                        lhsT=wo_b[1][:, mo * 128:(mo + 1) * 128],
                        rhs=out_ds[1][:, n:n + 512],
                        start=False, stop=True)
                for n in range(0, SQ, 256):
                    osb = sb2.tile([128, 256], F32, tag="osb", name="osb")
                    nc.vector.scalar_tensor_tensor(
                        osb[:], po[mo][:, n:n + 256], colc[mo][:, 0:1],
                        xq[mo][:, n:n + 256], OP.add, OP.add)
                    nc.sync.dma_start(
                        out=out_d[mo * 128:(mo + 1) * 128, n:n + 256],
                        in_=osb[:])

    _br.move_matmul_waits_to_ldweights(nc.m)
    _br.generate_event_semaphores(nc)
    return nc


# ---------------------------------------------------------------------------
def _consts():
    cidx = np.arange(C)
    pool = np.zeros((C, G), np.float32)
    pool[cidx, cidx // 8] = 1.0 / 8.0
    pooly = (pool * (8.0 / 32768.0)).astype(ml_dtypes.bfloat16)
    expand = np.zeros((G, C), np.float32)
    expand[cidx // 8, cidx] = 1.0
    return pool, pooly, expand


def make_in_maps(x, y, g1, b1, g2, b2, wq, bq, wk, bk, wv, bv, wo, bo):
    f = lambda a: np.ascontiguousarray(np.asarray(a, dtype=np.float32))
    bf = lambda a: np.ascontiguousarray(np.asarray(a).astype(ml_dtypes.bfloat16))
    x = f(x).reshape(2, C, S)
    y = f(y).reshape(2, C, S)
    xb16 = x.astype(ml_dtypes.bfloat16)
    yb16 = y.astype(ml_dtypes.bfloat16)
    pool, pooly, expand = _consts()
    bo2 = f(bo) + f(wo) @ f(bv)
    vecs = np.stack([f(bq) * (SCHRAUD_A / 8.0), bo2, f(g1), f(b1),
                     f(g2), f(b2)], axis=1).astype(np.float32)
    wallm = np.concatenate([f(wk).T, f(wv).T, f(wq).T * (SCHRAUD_A / 8.0),
                            f(wo).T], axis=1)
    base = {
        "wall": bf(wallm),
        "vp": np.ascontiguousarray(
            np.concatenate([vecs, pool], axis=1).astype(np.float32)),
        "poolym": pooly, "expandm": expand,
    }
    in_maps = []
    for core in range(8):
        b, sq = core // 4, core % 4
        m = dict(base)
        m["x"] = np.ascontiguousarray(xb16[b])
        m["y"] = np.ascontiguousarray(np.roll(yb16[b], -sq * SQ, axis=1))
        m["xq"] = np.ascontiguousarray(x[b][:, sq * SQ:(sq + 1) * SQ])
        in_maps.append(m)
    return in_maps


_NC_CACHE = None


def _get_nc():
    global _NC_CACHE
    if _NC_CACHE is None:
        _NC_CACHE = build_nc()
    return _NC_CACHE


def kernel(**inputs) -> np.ndarray:
    nc = _get_nc()
    in_maps = make_in_maps(**inputs)
    res = run_bass_kernel_spmd(nc, in_maps, core_ids=list(range(8)))
    out = np.empty((2, C, S), np.float32)
    for core in range(8):
        b, sq = core // 4, core % 4
        out[b][:, sq * SQ:(sq + 1) * SQ] = res.results[core]["out"]
    return out.reshape(2, C, 64, 64)


# revision 10
# speedup vs baseline: 1.0049x; 1.0049x over previous
"""MultiHeadAttnBlock TRN2 kernel, v4.

Sharding: core i = (batch b = i//4, query quarter sq = i%4).

Structure (engines in steady state):
 - PE: scores (K=64, two heads at row halves) + attn@v, 8 N=512 matmuls per
   key-tile; projections k/v interleaved so LDWEIGHTS hides under streams.
 - ACT: exp of score columns [0:512] of each head tile (table exp with the
   1/SCHRAUD_A descale in the free activation scale slot).
 - DVE: score columns [512:1024] via a Schraudolph bit-exp: scores arrive
   pre-scaled by SCHRAUD_A (folded into wq on host), so the op is a single
   round(x + B) into int16, bitcast to bf16.
 - sc PSUM is four one-bank tiles (per head x per 512-column chunk) and the
   e tile is split per engine, so each scores->exp->scores sub-pipeline
   cycles in ~960ns against a ~1750ns iteration - no WAR stalls.
 - group-norm affine is folded into the projection weights; k-bias is
   softmax-invariant (dropped), v-bias routed through wo into a final
   column, q-bias kept via a tiny wq@By matmul.
 - x stats on DVE bn_stats; y stats via ACT squares + PE pooling matmuls.
 - denominator: ones-column of v'; reciprocal via 32-lane DMA spread; the
   partition broadcast is a K=1 ones matmul into drained PSUM (no DRAM
   round trip).
"""

import numpy as np
import ml_dtypes

import concourse.bass as bass
import concourse.mybir as mybir
import bass_rust as _br
from concourse.tile import TileContext
from concourse.bass_utils import run_bass_kernel_spmd

F32 = mybir.dt.float32
BF16 = mybir.dt.bfloat16
I16 = mybir.dt.int16
AF = mybir.ActivationFunctionType
OP = mybir.AluOpType

C = 256
S = 4096
SQ = 1024
H = 4
D = 64
G = 32
EPS = 1e-6
NT = 32
VW = D + 2

SCHRAUD_A = 184.6650292
SCHRAUD_B = 16251.44
XA = 512   # exp column split: ACT [0:XA], DVE [XA:1024]


def build_nc():
    nc = bass.Bass("TRN2", target_bir_lowering=False, debug=False, num_devices=8)

    def din(name, shape, dt=F32):
        return nc.dram_tensor(name, shape, dt, kind="ExternalInput").ap()

    x_d = din("x", [C, S], BF16)
    y_d = din("y", [C, S], BF16)
    xq_d = din("xq", [C, SQ])
    wall_d = din("wall", [C, 4 * C], BF16)   # (wk|wv|wq|wo).T halves
    vp_d = din("vp", [C, 6 + G])             # (vecs|pool)
    pooly_d = din("poolym", [C, G], BF16)
    exp_d = din("expandm", [G, C])
    out_d = nc.dram_tensor("out", [C, SQ], F32, kind="ExternalOutput").ap()

    with TileContext(nc) as tc:
        with (
            tc.tile_pool(name="pers", bufs=1) as pers,
            tc.tile_pool(name="sb1", bufs=1) as sb1,
            tc.tile_pool(name="sb2", bufs=2) as sb2,
            tc.tile_pool(name="expp", bufs=2) as expp,
            tc.tile_pool(name="ps", bufs=1, space="PSUM") as ps,
        ):
            # ---- persistent tiles -------------------------------------
            xf = [pers.tile([128, S], BF16, tag=f"xf{m}", name=f"xf{m}")
                  for m in range(2)]
            yf = [pers.tile([128, S], BF16, tag=f"yf{m}", name=f"yf{m}")
                  for m in range(2)]
            xq = [pers.tile([128, SQ], F32, tag=f"xq{m}", name=f"xq{m}")
                  for m in range(2)]
            k_sb = [[pers.tile([128, 1024], BF16, tag=f"ksb{m}_{j}",
                               name=f"ksb{m}_{j}") for j in range(4)]
                    for m in range(2)]
            q_sb = [pers.tile([128, SQ], BF16, tag=f"qsb{m}", name=f"qsb{m}")
                    for m in range(2)]
            v_sb = [pers.tile([128, 8 * H * VW], BF16, tag=f"vsb{j}",
                              name=f"vsb{j}") for j in range(4)]
            out_ds = [pers.tile([128, SQ], BF16, tag=f"ods{m}", name=f"ods{m}")
                      for m in range(2)]
            wall = [pers.tile([128, 4 * C], BF16, tag=f"wall{m}",
                              name=f"wall{m}") for m in range(2)]
            wk_b = [wall[m][:, 0:C] for m in range(2)]
            wv_b = [wall[m][:, C:2 * C] for m in range(2)]
            wq_b = [wall[m][:, 2 * C:3 * C] for m in range(2)]
            wo_b = [wall[m][:, 3 * C:4 * C] for m in range(2)]
            wqf = [pers.tile([128, C], BF16, tag=f"wqf{m}", name=f"wqf{m}")
                   for m in range(2)]
            wkf = [pers.tile([128, C], BF16, tag=f"wkf{m}", name=f"wkf{m}")
                   for m in range(2)]
            wvf = [pers.tile([128, C], BF16, tag=f"wvf{m}", name=f"wvf{m}")
                   for m in range(2)]
            vp = [pers.tile([128, 6 + G], F32, tag=f"vp{m}", name=f"vp{m}")
                  for m in range(2)]
            _vc = {"bq8": 0, "bo2": 1, "g1": 2, "b1": 3, "g2": 4, "b2": 5}
            gb = {nm: [vp[m][:, i:i + 1] for m in range(2)]
                  for nm, i in _vc.items()}
            den32 = [pers.tile([32, 32], F32, tag=f"den32_{hh}",
                               name=f"den32_{hh}") for hh in range(2)]
            rc32 = [pers.tile([32, 32], F32, tag=f"rc32_{hh}",
                              name=f"rc32_{hh}") for hh in range(2)]
            rc32b = [pers.tile([32, 32], BF16, tag=f"rc32b_{hh}",
                               name=f"rc32b_{hh}") for hh in range(2)]
            rrow = [pers.tile([1, SQ], BF16, tag=f"rrow{hh}", name=f"rrow{hh}")
                    for hh in range(2)]
            ones1 = pers.tile([1, D], BF16, tag="ones1", name="ones1")
            colq = [pers.tile([128, 1], F32, tag=f"colq{m}", name=f"colq{m}")
                    for m in range(2)]
            colc = [pers.tile([128, 1], F32, tag=f"colc{m}", name=f"colc{m}")
                    for m in range(2)]
            bxb = [pers.tile([128, 1], BF16, tag=f"bxb{m}", name=f"bxb{m}")
                   for m in range(2)]
            byb = [pers.tile([128, 1], BF16, tag=f"byb{m}", name=f"byb{m}")
                   for m in range(2)]
            bvx_sb = [pers.tile([128, 1], BF16, tag=f"bvx{m}", name=f"bvx{m}")
                      for m in range(2)]
            pool_sb = [vp[m][:, 6:6 + G] for m in range(2)]
            pooly_sb = [pers.tile([128, G], BF16, tag=f"ply{m}",
                                  name=f"ply{m}") for m in range(2)]
            expand_sb = pers.tile([G, C], F32, tag="ex", name="ex")

            nc.gpsimd.memset(ones1[:], 1.0)
            for j in range(4):
                vview = v_sb[j][:].rearrange("p (t h e) -> p t h e", t=8, h=H)
                nc.gpsimd.memset(vview[:, :, :, D:D + 2], 1.0)

            # ---- stage 1: DMA + stats ---------------------------------
            s6x = [sb1.tile([128, 48], F32, tag=f"s6x{m}", name=f"s6x{m}")
                   for m in range(2)]
            # x first (8 chunk DMAs, issue cost ~0.6us each on Sync):
            # DVE bn_stats chases the DMA
            for m in range(2):
                cs = slice(m * 128, (m + 1) * 128)
                for ch in range(4):
                    fs = slice(ch * 1024, (ch + 1) * 1024)
                    nc.sync.dma_start(out=xf[m][:, fs], in_=x_d[cs, fs])
                    for h2 in range(2):
                        c8 = 2 * ch + h2
                        nc.vector.bn_stats(
                            s6x[m][:, c8 * 6:(c8 + 1) * 6],
                            xf[m][:, c8 * 512:(c8 + 1) * 512])
            # packed weights (needed at ~14us for the folds)
            for m in range(2):
                cs = slice(m * 128, (m + 1) * 128)
                for half in range(2):
                    nc.sync.dma_start(
                        out=wall[m][:, half * 512:(half + 1) * 512],
                        in_=wall_d[cs, half * 512:(half + 1) * 512])
            # small constants (tiny transfers; needed from ~12us)
            for m in range(2):
                nc.sync.dma_start(out=vp[m][:],
                                  in_=vp_d[m * 128:(m + 1) * 128, :])
                nc.sync.dma_start(out=pooly_sb[m][:],
                                  in_=pooly_d[m * 128:(m + 1) * 128, :])
            nc.sync.dma_start(out=expand_sb[:], in_=exp_d[:])
            # y second: ACT squares + PE pooling
            gy1 = ps.tile([G, 512], F32, tag="bA", padded_shape=[128, 512],
                          name="gy1")
            gy2 = ps.tile([G, 512], F32, tag="bB", padded_shape=[128, 512],
                          name="gy2")
            n1 = n2 = 0
            for ch in range(4):
                fs = slice(ch * 1024, (ch + 1) * 1024)
                for m in range(2):
                    cs = slice(m * 128, (m + 1) * 128)
                    nc.sync.dma_start(out=yf[m][:, fs], in_=y_d[cs, fs])
                for m in range(2):
                    for h2 in range(2):
                        f5 = slice(ch * 1024 + h2 * 512,
                                   ch * 1024 + (h2 + 1) * 512)
                        nc.tensor.matmul(gy1[:], lhsT=pooly_sb[m][:],
                                         rhs=yf[m][:, f5],
                                         start=(n1 == 0), stop=(n1 == 15))
                        n1 += 1
                        ysq = sb1.tile([128, 512], BF16,
                                       tag=f"ysq{n2 % 4}",
                                       name=f"ysqt{ch}_{m}_{h2}")
                        nc.scalar.square(ysq[:], yf[m][:, f5])
                        nc.tensor.matmul(gy2[:], lhsT=pooly_sb[m][:],
                                         rhs=ysq[:],
                                         start=(n2 == 0), stop=(n2 == 15))
                        n2 += 1
            for m in range(2):
                nc.sync.dma_start(out=xq[m][:],
                                  in_=xq_d[m * 128:(m + 1) * 128, :])

            def affine_tail(gs, gamma, beta, tagp):
                """gs [G,2] = (mu_g, E[x^2]_g) -> per-channel A,B [128,1]."""
                ve = nc.vector
                musq = sb1.tile([G, 1], F32, tag=f"gmusq{tagp}",
                                name=f"gmusq{tagp}")
                ve.tensor_mul(musq[:], gs[:, 0:1], gs[:, 0:1])
                veps = sb1.tile([G, 1], F32, tag=f"veps{tagp}",
                                name=f"veps{tagp}")
                ve.tensor_sub(veps[:], gs[:, 1:2], musq[:])
                ve.tensor_scalar_add(veps[:], veps[:], EPS)
                sq = sb1.tile([G, 1], F32, tag=f"gsq{tagp}", name=f"gsq{tagp}")
                nc.scalar.sqrt(sq[:], veps[:])
                r0 = sb1.tile([G, 1], F32, tag=f"gr0{tagp}", name=f"gr0{tagp}")
                nc.vector.reciprocal(r0[:], sq[:])
                y2 = sb1.tile([G, 1], F32, tag=f"gy2{tagp}", name=f"gy2{tagp}")
                ve.tensor_mul(y2[:], r0[:], r0[:])
                ve.tensor_mul(y2[:], veps[:], y2[:])
                ve.tensor_scalar(y2[:], y2[:], -0.5, 1.5, OP.mult, OP.add)
                gs2 = sb1.tile([G, 2], F32, tag=f"gs2{tagp}",
                               name=f"gs2{tagp}")
                ve.tensor_mul(gs2[:, 0:1], r0[:], y2[:])
                ve.tensor_copy(gs2[:, 1:2], gs[:, 0:1])
                A, B = [], []
                for m in range(2):
                    pc = ps.tile([128, 2], F32, tag="bD",
                                 padded_shape=[128, 512], name=f"pc{tagp}{m}")
                    nc.tensor.matmul(
                        pc[:], lhsT=expand_sb[:, m * 128:(m + 1) * 128],
                        rhs=gs2[:], start=True, stop=True)
                    a = sb1.tile([128, 1], F32, tag=f"A{tagp}{m}",
                                 name=f"A{tagp}{m}")
                    nc.vector.tensor_mul(a[:], pc[:, 0:1], gamma[m])
                    bmid = sb1.tile([128, 1], F32, tag=f"Bm{tagp}{m}",
                                    name=f"Bm{tagp}{m}")
                    nc.vector.tensor_mul(bmid[:], pc[:, 1:2], a[:])
                    b_ = sb1.tile([128, 1], F32, tag=f"B{tagp}{m}",
                                  name=f"B{tagp}{m}")
                    ve.tensor_sub(b_[:], beta[m], bmid[:])
                    A.append(a)
                    B.append(b_)
                return A, B

            # ---- x affine + folded wk/wv ------------------------------
            stats_c = []
            for m in range(2):
                mv = sb1.tile([128, 2], F32, tag=f"mvx{m}", name=f"mvx{m}")
                nc.vector.bn_aggr(mv[:], s6x[m][:])
                st = sb1.tile([128, 2], F32, tag=f"stx{m}", name=f"stx{m}")
                nc.vector.tensor_copy(st[:, 0:1], mv[:, 0:1])
                msq = sb1.tile([128, 1], F32, tag=f"msqx{m}", name=f"msqx{m}")
                nc.vector.tensor_mul(msq[:], mv[:, 0:1], mv[:, 0:1])
                nc.vector.tensor_add(st[:, 1:2], mv[:, 1:2], msq[:])
                stats_c.append(st)
            gpx = ps.tile([G, 2], F32, tag="bC", padded_shape=[128, 512],
                          name="gpx")
            for m in range(2):
                nc.tensor.matmul(gpx[:], lhsT=pool_sb[m][:],
                                 rhs=stats_c[m][:],
                                 start=(m == 0), stop=(m == 1))
            gsx = sb1.tile([G, 2], F32, tag="gsx", name="gsx")
            nc.vector.tensor_copy(gsx[:], gpx[:])
            Ax, Bx = affine_tail(gsx, gb["g1"], gb["b1"], "x")
            for m in range(2):
                nc.vector.tensor_scalar_mul(wkf[m][:], wk_b[m][:],
                                            Ax[m][:, 0:1])
                nc.vector.tensor_scalar_mul(wvf[m][:], wv_b[m][:],
                                            Ax[m][:, 0:1])
                nc.vector.tensor_copy(bxb[m][:], Bx[m][:])

            # ---- k + v projections, interleaved (LDW hiding) ----------
            kjobs = [("k", m, n) for m in range(2) for n in range(0, S, 512)]
            vjobs = [("v", t, 0) for t in range(NT)]
            order = []
            while kjobs or vjobs:
                if kjobs:
                    order.append(kjobs.pop(0))
                for _ in range(2):
                    if vjobs:
                        order.append(vjobs.pop(0))
            nk = nv = 0
            for job in order:
                if job[0] == "k":
                    _, m, n = job
                    pk = ps.tile([128, 512], F32,
                                 tag="bA" if nk % 2 == 0 else "bB",
                                 padded_shape=[128, 512], name=f"pk{m}_{n}")
                    for kk in range(2):
                        nc.tensor.matmul(
                            pk[:], lhsT=wkf[kk][:, m * 128:(m + 1) * 128],
                            rhs=xf[kk][:, n:n + 512],
                            start=(kk == 0), stop=(kk == 1))
                    kdst = k_sb[m][n // 1024][:, n % 1024:n % 1024 + 512]
                    if nk % 2 == 0:
                        nc.scalar.copy(kdst, pk[:])
                    else:
                        nc.vector.tensor_copy(kdst, pk[:])
                    nk += 1
                else:
                    _, t, _ = job
                    pv = ps.tile([128, C], F32,
                                 tag="bC" if nv % 2 == 0 else "bD",
                                 padded_shape=[128, 512], name=f"pv{t}")
                    tsl = slice(t * 128, (t + 1) * 128)
                    for kk in range(2):
                        nc.tensor.matmul(pv[:], lhsT=xf[kk][:, tsl],
                                         rhs=wvf[kk][:],
                                         start=(kk == 0), stop=(kk == 1))
                    pvv = pv[:].rearrange("p (h e) -> p h e", h=H)
                    dst = v_sb[t // 8][:, (t % 8) * H * VW:
                                       (t % 8 + 1) * H * VW]
                    dvv = dst.rearrange("p (h e) -> p h e", h=H)[:, :, 0:D]
                    if nv % 2 == 0:
                        nc.scalar.copy(dvv, pvv)
                    else:
                        nc.vector.tensor_copy(dvv, pvv)
                    nv += 1

            # ---- y affine + folded wq + q projection ------------------
            gsy = sb1.tile([G, 2], F32, tag="gsy", name="gsy")
            nc.vector.tensor_reduce(gsy[:, 0:1], gy1[:],
                                    mybir.AxisListType.X, OP.add)
            nc.vector.tensor_reduce(gsy[:, 1:2], gy2[:],
                                    mybir.AxisListType.X, OP.add)
            Ay, By = affine_tail(gsy, gb["g2"], gb["b2"], "y")
            for m in range(2):
                nc.vector.tensor_scalar_mul(wqf[m][:], wq_b[m][:],
                                            Ay[m][:, 0:1])
                nc.vector.tensor_copy(byb[m][:], By[m][:])
            pbq = [ps.tile([128, 1], F32, tag="bC", padded_shape=[128, 512],
                           name=f"pbq{mh}") for mh in range(2)]
            for mh in range(2):
                for kk in range(2):
                    nc.tensor.matmul(pbq[mh][:],
                                     lhsT=wq_b[kk][:, mh * 128:(mh + 1) * 128],
                                     rhs=byb[kk][:],
                                     start=(kk == 0), stop=(kk == 1))
                nc.vector.tensor_add(colq[mh][:], pbq[mh][:], gb["bq8"][mh])
            pqs = []
            for m in range(2):
                pq = ps.tile([128, SQ], F32, tag="dE" if m == 0 else "dF",
                             name=f"pq{m}")
                for n in range(0, SQ, 512):
                    for kk in range(2):
                        nc.tensor.matmul(
                            pq[:, n:n + 512],
                            lhsT=wqf[kk][:, m * 128:(m + 1) * 128],
                            rhs=yf[kk][:, n:n + 512],
                            start=(kk == 0), stop=(kk == 1))
                pqs.append(pq)
            for m in range(2):
                nc.vector.tensor_scalar_add(q_sb[m][:], pqs[m][:],
                                            colq[m][:, 0:1])

            # v bias chain: colc[mo] = wo@(wv@Bx) + bo2
            pbv = [ps.tile([128, 1], F32, tag="bC", padded_shape=[128, 512],
                           name=f"pbv{mh}") for mh in range(2)]
            for mh in range(2):
                for kk in range(2):
                    nc.tensor.matmul(pbv[mh][:],
                                     lhsT=wv_b[kk][:, mh * 128:(mh + 1) * 128],
                                     rhs=bxb[kk][:],
                                     start=(kk == 0), stop=(kk == 1))
                nc.vector.tensor_copy(bvx_sb[mh][:], pbv[mh][:])
            pbc = [ps.tile([128, 1], F32, tag="bD", padded_shape=[128, 512],
                           name=f"pbc{mo}") for mo in range(2)]
            for mo in range(2):
                for mh in range(2):
                    nc.tensor.matmul(pbc[mo][:],
                                     lhsT=wo_b[mh][:, mo * 128:(mo + 1) * 128],
                                     rhs=bvx_sb[mh][:],
                                     start=(mh == 0), stop=(mh == 1))
                nc.vector.tensor_add(colc[mo][:], pbc[mo][:], gb["bo2"][mo])

            # ---- stage 3: attention ------------------------------------
            po = []
            for p in range(2):
                sc = [[ps.tile([128, 512], F32,
                               tag=["bA", "bB", "bC", "bD"][2 * hh + ni],
                               padded_shape=[128, 512],
                               name=f"sc{p}_{hh}_{ni}")
                       for ni in range(2)] for hh in range(2)]
                acc = [ps.tile([VW, SQ], F32, tag=["dE", "dF"][hh],
                               padded_shape=[128, 1024],
                               name=f"acc{p}_{hh}") for hh in range(2)]

                def emit_scores(t):
                    tsl = slice((t % 8) * 128, (t % 8 + 1) * 128)
                    for ni in range(2):
                        for hh in range(2):
                            lo = hh * 64
                            n = ni * 512
                            nc.tensor.matmul(
                                sc[hh][ni][:],
                                lhsT=k_sb[p][t // 8][lo:lo + 64, tsl],
                                rhs=q_sb[p][lo:lo + 64, n:n + 512],
                                start=True, stop=True)

                def emit_exp(t):
                    es = []
                    for hh in range(2):
                        ea = expp.tile([128, XA], BF16, tag=f"ea{hh}",
                                       name=f"ea{p}_{hh}_{t}")
                        eb = expp.tile([128, SQ - XA], BF16, tag=f"eb{hh}",
                                       name=f"eb{p}_{hh}_{t}")
                        nc.scalar.activation(ea[:], sc[hh][0][:],
                                             AF.Exp, scale=1.0 / SCHRAUD_A)
                        nc.vector.tensor_scalar_add(
                            eb[:].bitcast(I16), sc[hh][1][:], SCHRAUD_B)
                        es.append((ea, eb))
                    return es

                emit_scores(0)
                for t in range(NT):
                    es = emit_exp(t)
                    if t + 1 < NT:
                        emit_scores(t + 1)
                    for hh in range(2):
                        h = 2 * p + hh
                        voff = (t % 8) * H * VW + h * VW
                        lhsv = v_sb[t // 8][:, voff:voff + VW]
                        for ni in range(2):
                            nc.tensor.matmul(
                                acc[hh][:, ni * 512:(ni + 1) * 512],
                                lhsT=lhsv, rhs=es[hh][ni][:],
                                start=(t == 0), stop=(t == NT - 1))
                # ---- drain ----
                asbs = []
                for hh in range(2):
                    asb = sb2.tile([VW, SQ], F32, tag="asb", name="asb")
                    if hh == 0:
                        nc.vector.tensor_copy(asb[:], acc[hh][:])
                    else:
                        nc.scalar.copy(asb[:], acc[hh][:])
                    asbs.append(asb)
                if p == 1:
                    # wo accumulation on out_ds[0] while the drain DMAs fly
                    # (reuses the just-freed acc banks)
                    for mo in range(2):
                        po_t = ps.tile([128, SQ], F32,
                                       tag="dE" if mo == 0 else "dF",
                                       name=f"po{mo}")
                        po.append(po_t)
                        for n in range(0, SQ, 512):
                            nc.tensor.matmul(
                                po_t[:, n:n + 512],
                                lhsT=wo_b[0][:, mo * 128:(mo + 1) * 128],
                                rhs=out_ds[0][:, n:n + 512],
                                start=True, stop=False)
                for hh in range(2):
                    # gather the 32-lane reciprocal into one bf16 row, then
                    # partition-broadcast via a K=1 ones matmul into drained
                    # PSUM.  Pair 0 uses the acc banks (dE/dF) so pair-1
                    # scores can claim bA..bD immediately.
                    nc.sync.dma_start(out=den32[hh][:],
                                      in_=asbs[hh][D:D + 1, :])
                    nc.vector.reciprocal(rc32[hh][:], den32[hh][:])
                    nc.vector.tensor_copy(rc32b[hh][:], rc32[hh][:])
                    nc.sync.dma_start(out=rrow[hh][:], in_=rc32b[hh][:])
                    if p == 0:
                        rbt = ps.tile([D, SQ], F32, tag=["dE", "dF"][hh],
                                      name=f"rb{p}{hh}")
                        rbs = [rbt[:, 0:512], rbt[:, 512:1024]]
                    else:
                        rbs = [ps.tile([D, 512], F32,
                                       tag=["bA", "bB", "bC", "bD"][2*hh+ni],
                                       padded_shape=[128, 512],
                                       name=f"rb{p}{hh}{ni}")[:]
                               for ni in range(2)]
                    for ni in range(2):
                        ns = slice(ni * 512, (ni + 1) * 512)
                        nc.tensor.matmul(rbs[ni], lhsT=ones1[:],
                                         rhs=rrow[hh][:, ns],
                                         start=True, stop=True)
                        if hh == 0:
                            nc.vector.tensor_mul(out_ds[p][0:64, ns],
                                                 asbs[hh][0:D, ns], rbs[ni])
                        else:
                            hsh = sb2.tile([64, 512], BF16, tag="hsh",
                                           name="hsh")
                            nc.vector.tensor_mul(hsh[:], asbs[hh][0:D, ns],
                                                 rbs[ni])
                            nc.sync.dma_start(out=out_ds[p][64:128, ns],
                                              in_=hsh[:])

            # ---- stage 4: output projection + residual -----------------
            for mo in range(2):
                for n in range(0, SQ, 512):
                    nc.tensor.matmul(
                        po[mo][:, n:n + 512],
                        lhsT=wo_b[1][:, mo * 128:(mo + 1) * 128],
                        rhs=out_ds[1][:, n:n + 512],
                        start=False, stop=True)
                for n in range(0, SQ, 256):
                    osb = sb2.tile([128, 256], F32, tag="osb", name="osb")
                    nc.vector.scalar_tensor_tensor(
                        osb[:], po[mo][:, n:n + 256], colc[mo][:, 0:1],
                        xq[mo][:, n:n + 256], OP.add, OP.add)
                    nc.sync.dma_start(
                        out=out_d[mo * 128:(mo + 1) * 128, n:n + 256],
                        in_=osb[:])

    _br.move_matmul_waits_to_ldweights(nc.m)
    _br.generate_event_semaphores(nc)
    return nc


# ---------------------------------------------------------------------------
def _consts():
    cidx = np.arange(C)
    pool = np.zeros((C, G), np.float32)
    pool[cidx, cidx // 8] = 1.0 / 8.0
    pooly = (pool * (8.0 / 32768.0)).astype(ml_dtypes.bfloat16)
    expand = np.zeros((G, C), np.float32)
    expand[cidx // 8, cidx] = 1.0
    return pool, pooly, expand


def make_in_maps(x, y, g1, b1, g2, b2, wq, bq, wk, bk, wv, bv, wo, bo):
    f = lambda a: np.ascontiguousarray(np.asarray(a, dtype=np.float32))
    bf = lambda a: np.ascontiguousarray(np.asarray(a).astype(ml_dtypes.bfloat16))
    x = f(x).reshape(2, C, S)
    y = f(y).reshape(2, C, S)
    xb16 = x.astype(ml_dtypes.bfloat16)
    yb16 = y.astype(ml_dtypes.bfloat16)
    pool, pooly, expand = _consts()
    bo2 = f(bo) + f(wo) @ f(bv)
    vecs = np.stack([f(bq) * (SCHRAUD_A / 8.0), bo2, f(g1), f(b1),
                     f(g2), f(b2)], axis=1).astype(np.float32)
    wallm = np.concatenate([f(wk).T, f(wv).T, f(wq).T * (SCHRAUD_A / 8.0),
                            f(wo).T], axis=1)
    base = {
        "wall": bf(wallm),
        "vp": np.ascontiguousarray(
            np.concatenate([vecs, pool], axis=1).astype(np.float32)),
        "poolym": pooly, "expandm": expand,
    }
    in_maps = []
    for core in range(8):
        b, sq = core // 4, core % 4
        m = dict(base)
        m["x"] = np.ascontiguousarray(xb16[b])
        m["y"] = np.ascontiguousarray(np.roll(yb16[b], -sq * SQ, axis=1))
        m["xq"] = np.ascontiguousarray(x[b][:, sq * SQ:(sq + 1) * SQ])
        in_maps.append(m)
    return in_maps


_NC_CACHE = None


def _get_nc():
    global _NC_CACHE
    if _NC_CACHE is None:
        _NC_CACHE = build_nc()
    return _NC_CACHE


def kernel(**inputs) -> np.ndarray:
    nc = _get_nc()
    in_maps = make_in_maps(**inputs)
    res = run_bass_kernel_spmd(nc, in_maps, core_ids=list(range(8)))
    out = np.empty((2, C, S), np.float32)
    for core in range(8):
        b, sq = core // 4, core % 4
        out[b][:, sq * SQ:(sq + 1) * SQ] = res.results[core]["out"]
    return out.reshape(2, C, 64, 64)


# revision 11
# speedup vs baseline: 1.0099x; 1.0049x over previous
"""MultiHeadAttnBlock TRN2 kernel, v4.

Sharding: core i = (batch b = i//4, query quarter sq = i%4).

Structure (engines in steady state):
 - PE: scores (K=64, two heads at row halves) + attn@v, 8 N=512 matmuls per
   key-tile; projections k/v interleaved so LDWEIGHTS hides under streams.
 - ACT: exp of score columns [0:512] of each head tile (table exp with the
   1/SCHRAUD_A descale in the free activation scale slot).
 - DVE: score columns [512:1024] via a Schraudolph bit-exp: scores arrive
   pre-scaled by SCHRAUD_A (folded into wq on host), so the op is a single
   round(x + B) into int16, bitcast to bf16.
 - sc PSUM is four one-bank tiles (per head x per 512-column chunk) and the
   e tile is split per engine, so each scores->exp->scores sub-pipeline
   cycles in ~960ns against a ~1750ns iteration - no WAR stalls.
 - group-norm affine is folded into the projection weights; k-bias is
   softmax-invariant (dropped), v-bias routed through wo into a final
   column, q-bias kept via a tiny wq@By matmul.
 - x stats on DVE bn_stats; y stats via ACT squares + PE pooling matmuls.
 - denominator: ones-column of v'; reciprocal via 32-lane DMA spread; the
   partition broadcast is a K=1 ones matmul into drained PSUM (no DRAM
   round trip).
"""

import numpy as np
import ml_dtypes

import concourse.bass as bass
import concourse.mybir as mybir
import bass_rust as _br
from concourse.tile import TileContext
from concourse.bass_utils import run_bass_kernel_spmd

F32 = mybir.dt.float32
BF16 = mybir.dt.bfloat16
I16 = mybir.dt.int16
AF = mybir.ActivationFunctionType
OP = mybir.AluOpType

C = 256
S = 4096
SQ = 1024
H = 4
D = 64
G = 32
EPS = 1e-6
NT = 32
VW = D + 2

SCHRAUD_A = 184.6650292
SCHRAUD_B = 16251.44
XA = 512   # exp column split: ACT [0:XA], DVE [XA:1024]


def build_nc():
    nc = bass.Bass("TRN2", target_bir_lowering=False, debug=False, num_devices=8)

    def din(name, shape, dt=F32):
        return nc.dram_tensor(name, shape, dt, kind="ExternalInput").ap()

    x_d = din("x", [C, S], BF16)
    y_d = din("y", [C, S], BF16)
    xq_d = din("xq", [C, SQ])
    wall_d = din("wall", [C, 4 * C], BF16)   # (wk|wv|wq|wo).T halves
    vp_d = din("vp", [C, 6 + G])             # (vecs|pool)
    pooly_d = din("poolym", [C, G], BF16)
    exp_d = din("expandm", [G, C])
    out_d = nc.dram_tensor("out", [C, SQ], F32, kind="ExternalOutput").ap()

    with TileContext(nc) as tc:
        with (
            tc.tile_pool(name="pers", bufs=1) as pers,
            tc.tile_pool(name="sb1", bufs=1) as sb1,
            tc.tile_pool(name="sb2", bufs=2) as sb2,
            tc.tile_pool(name="expp", bufs=2) as expp,
            tc.tile_pool(name="ps", bufs=1, space="PSUM") as ps,
        ):
            # ---- persistent tiles -------------------------------------
            xf = [pers.tile([128, S], BF16, tag=f"xf{m}", name=f"xf{m}")
                  for m in range(2)]
            yf = [pers.tile([128, S], BF16, tag=f"yf{m}", name=f"yf{m}")
                  for m in range(2)]
            xq = [pers.tile([128, SQ], F32, tag=f"xq{m}", name=f"xq{m}")
                  for m in range(2)]
            k_sb = [[pers.tile([128, 1024], BF16, tag=f"ksb{m}_{j}",
                               name=f"ksb{m}_{j}") for j in range(4)]
                    for m in range(2)]
            q_sb = [pers.tile([128, SQ], BF16, tag=f"qsb{m}", name=f"qsb{m}")
                    for m in range(2)]
            v_sb = [pers.tile([128, 8 * H * VW], BF16, tag=f"vsb{j}",
                              name=f"vsb{j}") for j in range(4)]
            out_ds = [pers.tile([128, SQ], BF16, tag=f"ods{m}", name=f"ods{m}")
                      for m in range(2)]
            wall = [pers.tile([128, 4 * C], BF16, tag=f"wall{m}",
                              name=f"wall{m}") for m in range(2)]
            wk_b = [wall[m][:, 0:C] for m in range(2)]
            wv_b = [wall[m][:, C:2 * C] for m in range(2)]
            wq_b = [wall[m][:, 2 * C:3 * C] for m in range(2)]
            wo_b = [wall[m][:, 3 * C:4 * C] for m in range(2)]
            wqf = [pers.tile([128, C], BF16, tag=f"wqf{m}", name=f"wqf{m}")
                   for m in range(2)]
            wkf = [pers.tile([128, C], BF16, tag=f"wkf{m}", name=f"wkf{m}")
                   for m in range(2)]
            wvf = [pers.tile([128, C], BF16, tag=f"wvf{m}", name=f"wvf{m}")
                   for m in range(2)]
            vp = [pers.tile([128, 6 + G], F32, tag=f"vp{m}", name=f"vp{m}")
                  for m in range(2)]
            _vc = {"bq8": 0, "bo2": 1, "g1": 2, "b1": 3, "g2": 4, "b2": 5}
            gb = {nm: [vp[m][:, i:i + 1] for m in range(2)]
                  for nm, i in _vc.items()}
            den32 = [pers.tile([32, 32], F32, tag=f"den32_{hh}",
                               name=f"den32_{hh}") for hh in range(2)]
            rc32 = [pers.tile([32, 32], F32, tag=f"rc32_{hh}",
                              name=f"rc32_{hh}") for hh in range(2)]
            rc32b = [pers.tile([32, 32], BF16, tag=f"rc32b_{hh}",
                               name=f"rc32b_{hh}") for hh in range(2)]
            rrow = [pers.tile([1, SQ], BF16, tag=f"rrow{hh}", name=f"rrow{hh}")
                    for hh in range(2)]
            ones1 = pers.tile([1, D], BF16, tag="ones1", name="ones1")
            colq = [pers.tile([128, 1], F32, tag=f"colq{m}", name=f"colq{m}")
                    for m in range(2)]
            colc = [pers.tile([128, 1], F32, tag=f"colc{m}", name=f"colc{m}")
                    for m in range(2)]
            bxb = [pers.tile([128, 1], BF16, tag=f"bxb{m}", name=f"bxb{m}")
                   for m in range(2)]
            byb = [pers.tile([128, 1], BF16, tag=f"byb{m}", name=f"byb{m}")
                   for m in range(2)]
            bvx_sb = [pers.tile([128, 1], BF16, tag=f"bvx{m}", name=f"bvx{m}")
                      for m in range(2)]
            pool_sb = [vp[m][:, 6:6 + G] for m in range(2)]
            pooly_sb = [pers.tile([128, G], BF16, tag=f"ply{m}",
                                  name=f"ply{m}") for m in range(2)]
            expand_sb = pers.tile([G, C], F32, tag="ex", name="ex")

            nc.gpsimd.memset(ones1[:], 1.0)
            for j in range(4):
                vview = v_sb[j][:].rearrange("p (t h e) -> p t h e", t=8, h=H)
                nc.gpsimd.memset(vview[:, :, :, D:D + 2], 1.0)

            # ---- stage 1: DMA + stats ---------------------------------
            s6x = [sb1.tile([128, 48], F32, tag=f"s6x{m}", name=f"s6x{m}")
                   for m in range(2)]
            # x first (8 chunk DMAs, issue cost ~0.6us each on Sync):
            # DVE bn_stats chases the DMA
            for m in range(2):
                cs = slice(m * 128, (m + 1) * 128)
                for ch in range(4):
                    fs = slice(ch * 1024, (ch + 1) * 1024)
                    nc.sync.dma_start(out=xf[m][:, fs], in_=x_d[cs, fs])
                    for h2 in range(2):
                        c8 = 2 * ch + h2
                        nc.vector.bn_stats(
                            s6x[m][:, c8 * 6:(c8 + 1) * 6],
                            xf[m][:, c8 * 512:(c8 + 1) * 512])
            # packed weights (needed at ~14us for the folds)
            for m in range(2):
                cs = slice(m * 128, (m + 1) * 128)
                for half in range(2):
                    nc.sync.dma_start(
                        out=wall[m][:, half * 512:(half + 1) * 512],
                        in_=wall_d[cs, half * 512:(half + 1) * 512])
            # small constants (tiny transfers; needed from ~12us)
            for m in range(2):
                nc.sync.dma_start(out=vp[m][:],
                                  in_=vp_d[m * 128:(m + 1) * 128, :])
                nc.sync.dma_start(out=pooly_sb[m][:],
                                  in_=pooly_d[m * 128:(m + 1) * 128, :])
            nc.sync.dma_start(out=expand_sb[:], in_=exp_d[:])
            # y second: ACT squares + PE pooling
            gy1 = ps.tile([G, 512], F32, tag="bA", padded_shape=[128, 512],
                          name="gy1")
            gy2 = ps.tile([G, 512], F32, tag="bB", padded_shape=[128, 512],
                          name="gy2")
            n1 = n2 = 0
            for ch in range(4):
                fs = slice(ch * 1024, (ch + 1) * 1024)
                for m in range(2):
                    cs = slice(m * 128, (m + 1) * 128)
                    nc.sync.dma_start(out=yf[m][:, fs], in_=y_d[cs, fs])
                for m in range(2):
                    for h2 in range(2):
                        f5 = slice(ch * 1024 + h2 * 512,
                                   ch * 1024 + (h2 + 1) * 512)
                        nc.tensor.matmul(gy1[:], lhsT=pooly_sb[m][:],
                                         rhs=yf[m][:, f5],
                                         start=(n1 == 0), stop=(n1 == 15))
                        n1 += 1
                        ysq = sb1.tile([128, 512], BF16,
                                       tag=f"ysq{n2 % 4}",
                                       name=f"ysqt{ch}_{m}_{h2}")
                        nc.scalar.square(ysq[:], yf[m][:, f5])
                        nc.tensor.matmul(gy2[:], lhsT=pooly_sb[m][:],
                                         rhs=ysq[:],
                                         start=(n2 == 0), stop=(n2 == 15))
                        n2 += 1
            for m in range(2):
                nc.sync.dma_start(out=xq[m][:],
                                  in_=xq_d[m * 128:(m + 1) * 128, :])

            def affine_tail(gs, gamma, beta, tagp):
                """gs [G,2] = (mu_g, E[x^2]_g) -> per-channel A,B [128,1]."""
                ve = nc.vector
                musq = sb1.tile([G, 1], F32, tag=f"gmusq{tagp}",
                                name=f"gmusq{tagp}")
                ve.tensor_mul(musq[:], gs[:, 0:1], gs[:, 0:1])
                veps = sb1.tile([G, 1], F32, tag=f"veps{tagp}",
                                name=f"veps{tagp}")
                ve.tensor_sub(veps[:], gs[:, 1:2], musq[:])
                ve.tensor_scalar_add(veps[:], veps[:], EPS)
                sq = sb1.tile([G, 1], F32, tag=f"gsq{tagp}", name=f"gsq{tagp}")
                nc.scalar.sqrt(sq[:], veps[:])
                r0 = sb1.tile([G, 1], F32, tag=f"gr0{tagp}", name=f"gr0{tagp}")
                nc.vector.reciprocal(r0[:], sq[:])
                y2 = sb1.tile([G, 1], F32, tag=f"gy2{tagp}", name=f"gy2{tagp}")
                ve.tensor_mul(y2[:], r0[:], r0[:])
                ve.tensor_mul(y2[:], veps[:], y2[:])
                ve.tensor_scalar(y2[:], y2[:], -0.5, 1.5, OP.mult, OP.add)
                gs2 = sb1.tile([G, 2], F32, tag=f"gs2{tagp}",
                               name=f"gs2{tagp}")
                ve.tensor_mul(gs2[:, 0:1], r0[:], y2[:])
                ve.tensor_copy(gs2[:, 1:2], gs[:, 0:1])
                A, B = [], []
                for m in range(2):
                    pc = ps.tile([128, 2], F32, tag="bD",
                                 padded_shape=[128, 512], name=f"pc{tagp}{m}")
                    nc.tensor.matmul(
                        pc[:], lhsT=expand_sb[:, m * 128:(m + 1) * 128],
                        rhs=gs2[:], start=True, stop=True)
                    a = sb1.tile([128, 1], F32, tag=f"A{tagp}{m}",
                                 name=f"A{tagp}{m}")
                    nc.vector.tensor_mul(a[:], pc[:, 0:1], gamma[m])
                    bmid = sb1.tile([128, 1], F32, tag=f"Bm{tagp}{m}",
                                    name=f"Bm{tagp}{m}")
                    nc.vector.tensor_mul(bmid[:], pc[:, 1:2], a[:])
                    b_ = sb1.tile([128, 1], F32, tag=f"B{tagp}{m}",
                                  name=f"B{tagp}{m}")
                    ve.tensor_sub(b_[:], beta[m], bmid[:])
                    A.append(a)
                    B.append(b_)
                return A, B

            # ---- x affine + folded wk/wv ------------------------------
            stats_c = []
            for m in range(2):
                mv = sb1.tile([128, 2], F32, tag=f"mvx{m}", name=f"mvx{m}")
                nc.vector.bn_aggr(mv[:], s6x[m][:])
                st = sb1.tile([128, 2], F32, tag=f"stx{m}", name=f"stx{m}")
                nc.vector.tensor_copy(st[:, 0:1], mv[:, 0:1])
                msq = sb1.tile([128, 1], F32, tag=f"msqx{m}", name=f"msqx{m}")
                nc.vector.tensor_mul(msq[:], mv[:, 0:1], mv[:, 0:1])
                nc.vector.tensor_add(st[:, 1:2], mv[:, 1:2], msq[:])
                stats_c.append(st)
            gpx = ps.tile([G, 2], F32, tag="bC", padded_shape=[128, 512],
                          name="gpx")
            for m in range(2):
                nc.tensor.matmul(gpx[:], lhsT=pool_sb[m][:],
                                 rhs=stats_c[m][:],
                                 start=(m == 0), stop=(m == 1))
            gsx = sb1.tile([G, 2], F32, tag="gsx", name="gsx")
            nc.vector.tensor_copy(gsx[:], gpx[:])
            Ax, Bx = affine_tail(gsx, gb["g1"], gb["b1"], "x")
            for m in range(2):
                nc.vector.tensor_scalar_mul(wkf[m][:], wk_b[m][:],
                                            Ax[m][:, 0:1])
                nc.vector.tensor_scalar_mul(wvf[m][:], wv_b[m][:],
                                            Ax[m][:, 0:1])
                nc.vector.tensor_copy(bxb[m][:], Bx[m][:])

            # ---- k + v projections, interleaved (LDW hiding) ----------
            kjobs = [("k", m, n) for m in range(2) for n in range(0, S, 512)]
            vjobs = [("v", t, 0) for t in range(NT)]
            order = []
            while kjobs or vjobs:
                if kjobs:
                    order.append(kjobs.pop(0))
                for _ in range(2):
                    if vjobs:
                        order.append(vjobs.pop(0))
            nk = nv = 0
            for job in order:
                if job[0] == "k":
                    _, m, n = job
                    pk = ps.tile([128, 512], F32,
                                 tag="bA" if nk % 2 == 0 else "bB",
                                 padded_shape=[128, 512], name=f"pk{m}_{n}")
                    for kk in range(2):
                        nc.tensor.matmul(
                            pk[:], lhsT=wkf[kk][:, m * 128:(m + 1) * 128],
                            rhs=xf[kk][:, n:n + 512],
                            start=(kk == 0), stop=(kk == 1))
                    kdst = k_sb[m][n // 1024][:, n % 1024:n % 1024 + 512]
                    if nk % 2 == 0:
                        nc.scalar.copy(kdst, pk[:])
                    else:
                        nc.vector.tensor_copy(kdst, pk[:])
                    nk += 1
                else:
                    _, t, _ = job
                    pv = ps.tile([128, C], F32,
                                 tag="bC" if nv % 2 == 0 else "bD",
                                 padded_shape=[128, 512], name=f"pv{t}")
                    tsl = slice(t * 128, (t + 1) * 128)
                    for kk in range(2):
                        nc.tensor.matmul(pv[:], lhsT=xf[kk][:, tsl],
                                         rhs=wvf[kk][:],
                                         start=(kk == 0), stop=(kk == 1))
                    pvv = pv[:].rearrange("p (h e) -> p h e", h=H)
                    dst = v_sb[t // 8][:, (t % 8) * H * VW:
                                       (t % 8 + 1) * H * VW]
                    dvv = dst.rearrange("p (h e) -> p h e", h=H)[:, :, 0:D]
                    if nv % 2 == 0:
                        nc.scalar.copy(dvv, pvv)
                    else:
                        nc.vector.tensor_copy(dvv, pvv)
                    nv += 1

            # ---- y affine + folded wq + q projection ------------------
            gsy = sb1.tile([G, 2], F32, tag="gsy", name="gsy")
            nc.vector.tensor_reduce(gsy[:, 0:1], gy1[:],
                                    mybir.AxisListType.X, OP.add)
            nc.vector.tensor_reduce(gsy[:, 1:2], gy2[:],
                                    mybir.AxisListType.X, OP.add)
            Ay, By = affine_tail(gsy, gb["g2"], gb["b2"], "y")
            for m in range(2):
                nc.vector.tensor_scalar_mul(wqf[m][:], wq_b[m][:],
                                            Ay[m][:, 0:1])
                nc.vector.tensor_copy(byb[m][:], By[m][:])
            pbq = [ps.tile([128, 1], F32, tag="bC", padded_shape=[128, 512],
                           name=f"pbq{mh}") for mh in range(2)]
            for mh in range(2):
                for kk in range(2):
                    nc.tensor.matmul(pbq[mh][:],
                                     lhsT=wq_b[kk][:, mh * 128:(mh + 1) * 128],
                                     rhs=byb[kk][:],
                                     start=(kk == 0), stop=(kk == 1))
                nc.vector.tensor_add(colq[mh][:], pbq[mh][:], gb["bq8"][mh])
            pqs = []
            for m in range(2):
                pq = ps.tile([128, SQ], F32, tag="dE" if m == 0 else "dF",
                             name=f"pq{m}")
                for n in range(0, SQ, 512):
                    for kk in range(2):
                        nc.tensor.matmul(
                            pq[:, n:n + 512],
                            lhsT=wqf[kk][:, m * 128:(m + 1) * 128],
                            rhs=yf[kk][:, n:n + 512],
                            start=(kk == 0), stop=(kk == 1))
                pqs.append(pq)
            for m in range(2):
                nc.vector.tensor_scalar_add(q_sb[m][:], pqs[m][:],
                                            colq[m][:, 0:1])

            # v bias chain: colc[mo] = wo@(wv@Bx) + bo2
            pbv = [ps.tile([128, 1], F32, tag="bC", padded_shape=[128, 512],
                           name=f"pbv{mh}") for mh in range(2)]
            for mh in range(2):
                for kk in range(2):
                    nc.tensor.matmul(pbv[mh][:],
                                     lhsT=wv_b[kk][:, mh * 128:(mh + 1) * 128],
                                     rhs=bxb[kk][:],
                                     start=(kk == 0), stop=(kk == 1))
                nc.vector.tensor_copy(bvx_sb[mh][:], pbv[mh][:])
            pbc = [ps.tile([128, 1], F32, tag="bD", padded_shape=[128, 512],
                           name=f"pbc{mo}") for mo in range(2)]
            for mo in range(2):
                for mh in range(2):
                    nc.tensor.matmul(pbc[mo][:],
                                     lhsT=wo_b[mh][:, mo * 128:(mo + 1) * 128],
                                     rhs=bvx_sb[mh][:],
                                     start=(mh == 0), stop=(mh == 1))
                nc.vector.tensor_add(colc[mo][:], pbc[mo][:], gb["bo2"][mo])

            # ---- stage 3: attention ------------------------------------
            po = []
            for p in range(2):
                sc = [[ps.tile([128, 512], F32,
                               tag=["bA", "bB", "bC", "bD"][2 * hh + ni],
                               padded_shape=[128, 512],
                               name=f"sc{p}_{hh}_{ni}")
                       for ni in range(2)] for hh in range(2)]
                acc = [ps.tile([VW, SQ], F32, tag=["dE", "dF"][hh],
                               padded_shape=[128, 1024],
                               name=f"acc{p}_{hh}") for hh in range(2)]

                def emit_scores(t):
                    tsl = slice((t % 8) * 128, (t % 8 + 1) * 128)
                    for ni in range(2):
                        for hh in range(2):
                            lo = hh * 64
                            n = ni * 512
                            nc.tensor.matmul(
                                sc[hh][ni][:],
                                lhsT=k_sb[p][t // 8][lo:lo + 64, tsl],
                                rhs=q_sb[p][lo:lo + 64, n:n + 512],
                                start=True, stop=True)

                def emit_exp(t):
                    es = []
                    for hh in range(2):
                        ea = expp.tile([128, XA], BF16, tag=f"ea{hh}",
                                       name=f"ea{p}_{hh}_{t}")
                        eb = expp.tile([128, SQ - XA], BF16, tag=f"eb{hh}",
                                       name=f"eb{p}_{hh}_{t}")
                        nc.scalar.activation(ea[:], sc[hh][0][:],
                                             AF.Exp, scale=1.0 / SCHRAUD_A)
                        nc.vector.tensor_scalar_add(
                            eb[:].bitcast(I16), sc[hh][1][:], SCHRAUD_B)
                        es.append((ea, eb))
                    return es

                emit_scores(0)
                for t in range(NT):
                    es = emit_exp(t)
                    if t + 1 < NT:
                        emit_scores(t + 1)
                    for hh in range(2):
                        h = 2 * p + hh
                        voff = (t % 8) * H * VW + h * VW
                        lhsv = v_sb[t // 8][:, voff:voff + VW]
                        for ni in range(2):
                            nc.tensor.matmul(
                                acc[hh][:, ni * 512:(ni + 1) * 512],
                                lhsT=lhsv, rhs=es[hh][ni][:],
                                start=(t == 0), stop=(t == NT - 1))
                # ---- drain ----
                asbs = []
                for hh in range(2):
                    asb = sb2.tile([VW, SQ], F32, tag="asb", name="asb")
                    if hh == 0:
                        nc.vector.tensor_copy(asb[:], acc[hh][:])
                    else:
                        nc.scalar.copy(asb[:], acc[hh][:])
                    asbs.append(asb)
                if p == 1:
                    # wo accumulation on out_ds[0] while the drain DMAs fly
                    # (reuses the just-freed acc banks)
                    for mo in range(2):
                        po_t = ps.tile([128, SQ], F32,
                                       tag="dE" if mo == 0 else "dF",
                                       name=f"po{mo}")
                        po.append(po_t)
                        for n in range(0, SQ, 512):
                            nc.tensor.matmul(
                                po_t[:, n:n + 512],
                                lhsT=wo_b[0][:, mo * 128:(mo + 1) * 128],
                                rhs=out_ds[0][:, n:n + 512],
                                start=True, stop=False)
                for hh in range(2):
                    # gather the 32-lane reciprocal into one bf16 row, then
                    # partition-broadcast via a K=1 ones matmul into drained
                    # PSUM.  Pair 0 uses the acc banks (dE/dF) so pair-1
                    # scores can claim bA..bD immediately.
                    nc.sync.dma_start(out=den32[hh][:],
                                      in_=asbs[hh][D:D + 1, :])
                    nc.vector.reciprocal(rc32[hh][:], den32[hh][:])
                    nc.vector.tensor_copy(rc32b[hh][:], rc32[hh][:])
                    nc.sync.dma_start(out=rrow[hh][:], in_=rc32b[hh][:])
                    if p == 0:
                        rbt = ps.tile([D, SQ], F32, tag=["dE", "dF"][hh],
                                      name=f"rb{p}{hh}")
                        rbs = [rbt[:, 0:512], rbt[:, 512:1024]]
                    else:
                        rbs = [ps.tile([D, 512], F32,
                                       tag=["bA", "bB", "bC", "bD"][2*hh+ni],
                                       padded_shape=[128, 512],
                                       name=f"rb{p}{hh}{ni}")[:]
                               for ni in range(2)]
                    for ni in range(2):
                        ns = slice(ni * 512, (ni + 1) * 512)
                        nc.tensor.matmul(rbs[ni], lhsT=ones1[:],
                                         rhs=rrow[hh][:, ns],
                                         start=True, stop=True)
                        if hh == 0:
                            nc.vector.tensor_mul(out_ds[p][0:64, ns],
                                                 asbs[hh][0:D, ns], rbs[ni])
                        else:
                            hsh = sb2.tile([64, 512], BF16, tag="hsh",
                                           name="hsh")
                            nc.vector.tensor_mul(hsh[:], asbs[hh][0:D, ns],
                                                 rbs[ni])
                            nc.sync.dma_start(out=out_ds[p][64:128, ns],
                                              in_=hsh[:])

            # ---- stage 4: output projection + residual -----------------
            for mo in range(2):
                for n in range(0, SQ, 512):
                    nc.tensor.matmul(
                        po[mo][:, n:n + 512],
                        lhsT=wo_b[1][:, mo * 128:(mo + 1) * 128],
                        rhs=out_ds[1][:, n:n + 512],
                        start=False, stop=True)
                for n in range(0, SQ, 512):
                    osb = sb2.tile([128, 512], F32, tag="osb", name="osb")
                    nc.vector.scalar_tensor_tensor(
                        osb[:], po[mo][:, n:n + 512], colc[mo][:, 0:1],
                        xq[mo][:, n:n + 512], OP.add, OP.add)
                    nc.sync.dma_start(
                        out=out_d[mo * 128:(mo + 1) * 128, n:n + 512],
                        in_=osb[:])

    _br.move_matmul_waits_to_ldweights(nc.m)
    _br.generate_event_semaphores(nc)
    return nc


# ---------------------------------------------------------------------------
def _consts():
    cidx = np.arange(C)
    pool = np.zeros((C, G), np.float32)
    pool[cidx, cidx // 8] = 1.0 / 8.0
    pooly = (pool * (8.0 / 32768.0)).astype(ml_dtypes.bfloat16)
    expand = np.zeros((G, C), np.float32)
    expand[cidx // 8, cidx] = 1.0
    return pool, pooly, expand


def make_in_maps(x, y, g1, b1, g2, b2, wq, bq, wk, bk, wv, bv, wo, bo):
    f = lambda a: np.ascontiguousarray(np.asarray(a, dtype=np.float32))
    bf = lambda a: np.ascontiguousarray(np.asarray(a).astype(ml_dtypes.bfloat16))
    x = f(x).reshape(2, C, S)
    y = f(y).reshape(2, C, S)
    xb16 = x.astype(ml_dtypes.bfloat16)
    yb16 = y.astype(ml_dtypes.bfloat16)
    pool, pooly, expand = _consts()
    bo2 = f(bo) + f(wo) @ f(bv)
    vecs = np.stack([f(bq) * (SCHRAUD_A / 8.0), bo2, f(g1), f(b1),
                     f(g2), f(b2)], axis=1).astype(np.float32)
    wallm = np.concatenate([f(wk).T, f(wv).T, f(wq).T * (SCHRAUD_A / 8.0),
                            f(wo).T], axis=1)
    base = {
        "wall": bf(wallm),
        "vp": np.ascontiguousarray(
            np.concatenate([vecs, pool], axis=1).astype(np.float32)),
        "poolym": pooly, "expandm": expand,
    }
    in_maps = []
    for core in range(8):
        b, sq = core // 4, core % 4
        m = dict(base)
        m["x"] = np.ascontiguousarray(xb16[b])
        m["y"] = np.ascontiguousarray(np.roll(yb16[b], -sq * SQ, axis=1))
        m["xq"] = np.ascontiguousarray(x[b][:, sq * SQ:(sq + 1) * SQ])
        in_maps.append(m)
    return in_maps


_NC_CACHE = None


def _get_nc():
    global _NC_CACHE
    if _NC_CACHE is None:
        _NC_CACHE = build_nc()
    return _NC_CACHE


def kernel(**inputs) -> np.ndarray:
    nc = _get_nc()
    in_maps = make_in_maps(**inputs)
    res = run_bass_kernel_spmd(nc, in_maps, core_ids=list(range(8)))
    out = np.empty((2, C, S), np.float32)
    for core in range(8):
        b, sq = core // 4, core % 4
        out[b][:, sq * SQ:(sq + 1) * SQ] = res.results[core]["out"]
    return out.reshape(2, C, 64, 64)
